# revision 5
# baseline (speedup 1.0000x reference)
"""Trainium2 Bass kernel for nn_KnowledgeRetriever (retrieval_knn).

Reference semantics:
    q = normalize(query_flat); kn = normalize(knowledge)
    sim = q @ kn.T                        # [B*S, K]
    top_k = argsort(sim)[..., -K:]        # K == max_chunks == 64 -> ALL indices
    out = mean(knowledge[top_k], axis=1)  # mean over a permutation of all rows

Because top_k is always a full permutation of range(K), the mean is
permutation-invariant: out[b, s, :] == knowledge.mean(axis=0) for every
(b, s). The similarity/argsort/gather pipeline is dead code. The kernel
therefore computes the column mean of knowledge on-device (one matmul
against a 1/K constant) and broadcasts it into the [B*S, E] output.

Sharding: data-parallel over the flattened B*S=4096 query rows; each of
the 8 cores writes its 512-row output slice. knowledge is replicated.
"""

import numpy as np

import concourse.bass as bass
from concourse import mybir
from concourse.bass_utils import run_bass_kernel_spmd

B, S, E = 4, 1024, 512
K = 64
N_CORES = 8
ROWS_PER_CORE = (B * S) // N_CORES  # 512
P = 128  # SBUF partitions

_CACHE: dict = {}


def _build() -> bass.Bass:
    nc = bass.Bass("TRN2", debug=False, target_bir_lowering=False,
                   num_devices=N_CORES)
    kn = nc.dram_tensor("knowledge", [K, E], mybir.dt.float32,
                        kind="ExternalInput")
    out = nc.dram_tensor("out", [ROWS_PER_CORE, E], mybir.dt.float32,
                         kind="ExternalOutput")

    n_out_tiles = ROWS_PER_CORE // P  # 4

    with (
        nc.semaphore("w_sem") as w_sem,
        nc.semaphore("dma_sem") as dma_sem,
        nc.semaphore("mm_sem") as mm_sem,
        nc.semaphore("cp_sem") as cp_sem,
        nc.sbuf_tensor("w_mean", [K, P], mybir.dt.float32) as w_mean,
        nc.sbuf_tensor("ktile", [K, E], mybir.dt.float32) as ktile,
        nc.psum_tensor("pmean", [P, E], mybir.dt.float32) as pmean,
        nc.sbuf_tensor("bcast", [P, E], mybir.dt.float32) as bcast,
    ):
        with nc.Block() as block:

            @block.gpsimd
            def _(gpsimd):
                # lhsT[K, P] of 1/K: out[p, e] = sum_k knowledge[k, e] / K
                # -> every output partition holds the mean row.
                gpsimd.memset(w_mean.ap(), 1.0 / K).then_inc(w_sem, 1)

            @block.sync
            def _(sync):
                sync.dma_start(out=ktile.ap(), in_=kn.ap()).then_inc(dma_sem, 16)

            @block.tensor
            def _(tensor):
                tensor.wait_ge(dma_sem, 16)
                tensor.wait_ge(w_sem, 1)
                tensor.matmul(pmean.ap(), w_mean.ap(), ktile.ap(),
                              start=True, stop=True).then_inc(mm_sem, 1)

            @block.vector
            def _(vector):
                vector.wait_ge(mm_sem, 1)
                vector.tensor_copy(out=bcast.ap(), in_=pmean.ap()).then_inc(
                    cp_sem, 1)

            @block.sync
            def _(sync):
                sync.wait_ge(cp_sem, 1)
                for i in range(n_out_tiles):
                    sync.dma_start(
                        out=out.ap()[i * P:(i + 1) * P, :], in_=bcast.ap()
                    ).then_inc(dma_sem, 16)
                sync.wait_ge(dma_sem, 16 * (1 + n_out_tiles))
    return nc


def run(knowledge: np.ndarray, trace: bool = False, tmpdir: str | None = None):
    """Dispatch to the 8 cores; returns (full [B,S,E] output, BassKernelResults)."""
    if "nc" not in _CACHE:
        _CACHE["nc"] = _build()
    nc = _CACHE["nc"]
    kn = np.ascontiguousarray(np.asarray(knowledge, dtype=np.float32))
    in_maps = [{"knowledge": kn} for _ in range(N_CORES)]
    res = run_bass_kernel_spmd(nc, in_maps, list(range(N_CORES)), trace=trace,
                               tmpdir=tmpdir)
    full = np.concatenate([res.results[c]["out"] for c in range(N_CORES)],
                          axis=0).reshape(B, S, E)
    return full, res


def kernel(query_embedding: np.ndarray, knowledge: np.ndarray) -> np.ndarray:
    # query_embedding only selects the permutation order inside the dead
    # argsort/gather path; the output does not depend on its values.
    full, _ = run(knowledge, trace=False)
    return full
